# revision 16
# baseline (speedup 1.0000x reference)
"""Trainium2 Bass kernel for the emoji-box decoder problem (optimized v5).

Math: per picture, softmax(-d2) over emoji pixels is separable:
softmax_r (x) softmax_c.  With A = t1^T AcT the unnormalized row sum
(AcT column-normalized and column-masked), the output satisfies

    res = z' * (A - Zr (x) colin) + valid
    z'[p] = valid * rowin[p] * (1/Zr[p]),

because res = valid*(rowin*colin*(R-1) + 1) = where(valid,
where(inside, R, 1), 0).  The -Zr (x) colin term is ACCUMULATED INTO
THE SAME PSUM BANK as the eR matmul via a k=1 outer-product matmul
(lhsT = -Zr row in bf16, rhs = colin row), so the whole blend collapses
to ONE fused op per output group (tensor_scalar on vector for ch0/1,
Identity-activation with scale/bias pointers on scalar for ch2).
Zr is produced BOTH as a row (ones^T ErT) and as a column (ErT^T ones)
by two tiny PE matmuls off the same ErT table.  Both Ac tiles are
transposed in ONE [128,128] PE pass.

Engine split respecting TRN2 ISA limits (Pool supports no float
elementwise; scalar does fused affine activations with per-partition
scale/bias pointers):
  vector: argmax, box coords, masks, softmax normalize, Ac tiles,
          psum copies (AcT half 0, t1 ch0/1), z' chain, res_ab tail
  scalar: Square distance grids (fused affine), one big exp, ErT
          chain, -Zr row, AcT half 1 + t1 ch2 copies, res_c tail
  gpsimd: iotas / memsets / identity matrix only
  PE: one transpose, Zr row+col matmuls, correction outers, t1
      (ch2 first), eR (c before ab)

Emoji selection: argmax of logits -> sync register -> dynamic-offset
DRAM->SBUF DMA gather of the selected emoji (bf16, i-major layout so
each partition reads one contiguous 384B chunk).

Sharding: 8 cores = 2 pictures x 4 row-blocks of 64 canvas rows.
"""

import sys

import numpy as np

if "/opt/trn_rl_repo" not in sys.path:
    sys.path.insert(0, "/opt/trn_rl_repo")

import ml_dtypes

import concourse.bacc as bacc
import concourse.bass as bass
import concourse.mybir as mybir
import concourse.tile as tile
from concourse.bass_utils import run_bass_kernel_spmd


def _ensure_ntff_hook():
    """The image's antenv package lacks axon_hooks, so trn_boot's NTFF
    profile hook install degrades silently and run_bass_kernel_spmd
    crashes on `from antenv.axon_hooks import ...` when trace=True.
    Provide the module and install the ctypes hook ourselves."""
    import types

    try:
        from antenv.axon_hooks import get_axon_ntff_profile_hook  # noqa: F401

        return
    except ImportError:
        pass
    mod = types.ModuleType("antenv.axon_hooks")
    _hook = [None]
    mod.set_axon_ntff_profile_hook = lambda h: _hook.__setitem__(0, h)
    mod.get_axon_ntff_profile_hook = lambda: _hook[0]
    try:
        import antenv

        sys.modules["antenv.axon_hooks"] = mod
        antenv.axon_hooks = mod
        from trn_agent_boot.trn_boot import _ntff_profile_via_ctypes

        hook = _ntff_profile_via_ctypes("/opt/axon/libaxon_pjrt.so")
        if hook is not None:
            mod.set_axon_ntff_profile_hook(hook)
    except Exception:
        pass


_ensure_ntff_hook()

F32 = mybir.dt.float32
BF16 = mybir.dt.bfloat16
I32 = mybir.dt.int32
AF = mybir.ActivationFunctionType
OP = mybir.AluOpType
AX = mybir.AxisListType

MAGIC = 8388608.0  # 2**23; x + MAGIC - MAGIC == rint(x) for 0 <= x < 2**22

N_CORES = 8
H = 256
S = 64
N_IMG = 14
RB = 64  # canvas rows per core


def build_nc():
    nc = bacc.Bacc("TRN2", target_bir_lowering=False, debug=False)

    xmeta_d = nc.dram_tensor("xmeta", [128, 24], F32, kind="ExternalInput")
    imgs_d = nc.dram_tensor("imgs", [N_IMG, S, 3 * S], BF16, kind="ExternalInput")
    out_d = nc.dram_tensor("out", [3, RB, H], F32, kind="ExternalOutput")

    with tile.TileContext(nc) as tc:
        with (
            tc.tile_pool(name="constp", bufs=1) as constp,
            tc.tile_pool(name="workp", bufs=2) as workp,
            tc.tile_pool(name="outp", bufs=1) as outp,
            tc.tile_pool(name="ps_a", bufs=1, space="PSUM") as ps_a,
            tc.tile_pool(name="ps_z", bufs=1, space="PSUM") as ps_z,
            tc.tile_pool(name="ps_t1", bufs=1, space="PSUM") as ps_t1,
            tc.tile_pool(name="ps_r", bufs=1, space="PSUM") as ps_r,
            tc.tile_pool(name="ps_c", bufs=1, space="PSUM") as ps_c,
        ):
            # ---- single input DMA on the sync HWDGE ring
            xb = constp.tile([128, 24], F32)
            nc.sync.dma_start(xb[:], xmeta_d[:])

            # ---- warm the scalar-engine activation table early so the
            # ~1.3us ACT_TABLE_LOAD overlaps the input DMA
            warm = workp.tile([1, 1], F32)
            nc.gpsimd.memset(warm[:], 0.0)
            warm2 = workp.tile([1, 1], F32)
            nc.scalar.activation(warm2[:], warm[:], AF.Exp)

            # ---- constants / iotas (gpsimd+vector, overlap the DMA)
            iota_pi = constp.tile([128, 1], I32)
            nc.gpsimd.iota(iota_pi[:], pattern=[[1, 1]], base=0, channel_multiplier=1)
            iota_pf = constp.tile([128, 1], F32)
            nc.vector.tensor_copy(iota_pf[:], iota_pi[:])
            iota14i = constp.tile([1, N_IMG], I32)
            nc.gpsimd.iota(iota14i[:], pattern=[[1, N_IMG]], base=0, channel_multiplier=0)
            iota14f = constp.tile([1, N_IMG], F32)
            nc.vector.tensor_copy(iota14f[:], iota14i[:])
            iota64i = constp.tile([128, 64], I32)
            nc.gpsimd.iota(iota64i[:], pattern=[[1, 64]], base=0, channel_multiplier=0)
            iota64f = constp.tile([128, 64], F32)
            nc.vector.tensor_copy(iota64f[:], iota64i[:])
            iota256i = constp.tile([1, 256], I32)
            nc.gpsimd.iota(iota256i[:], pattern=[[1, 256]], base=0, channel_multiplier=0)
            iota256f = constp.tile([1, 256], F32)
            nc.vector.tensor_copy(iota256f[:], iota256i[:])
            # iotap2[p, t] = p + 128*t  (both canvas-column tiles at once)
            iotap2i = constp.tile([128, 2], I32)
            nc.gpsimd.iota(iotap2i[:], pattern=[[128, 2]], base=0, channel_multiplier=1)
            iotap2 = constp.tile([128, 2], F32)
            nc.vector.tensor_copy(iotap2[:], iotap2i[:])
            # pm64[p] = p % 64 via two sliced iotas (channel index is
            # relative to the AP start)
            pm64i = constp.tile([128, 1], I32)
            nc.gpsimd.iota(pm64i[0:64, :], pattern=[[1, 1]], base=0, channel_multiplier=1)
            nc.gpsimd.iota(pm64i[64:128, :], pattern=[[1, 1]], base=0, channel_multiplier=1)
            pm64 = constp.tile([128, 1], F32)
            nc.vector.tensor_copy(pm64[:], pm64i[:])
            c64 = constp.tile([128, 1], F32)
            nc.vector.memset(c64[:], 1.0 / 64.0)
            ones64_bf = constp.tile([64, 1], BF16)
            nc.gpsimd.memset(ones64_bf[:], 1.0)
            onebf = workp.tile([128, 128], BF16)
            nc.gpsimd.memset(onebf[:], 1.0)
            idbf = constp.tile([128, 128], BF16)
            nc.gpsimd.affine_select(
                idbf[:],
                onebf[:],
                pattern=[[1, 128]],
                compare_op=OP.is_equal,
                fill=0.0,
                base=0,
                channel_multiplier=-1,
            )

            # ================= after xmeta arrives =================
            # ---- emoji index chain (vector) -> sync register -> gather
            rmax = workp.tile([1, 1], F32)
            nc.vector.tensor_reduce(rmax[:], xb[0:1, 5:19], AX.X, OP.max)
            dotj = workp.tile([1, N_IMG], I32)
            nc.vector.scalar_tensor_tensor(
                dotj[:], xb[0:1, 5:19], rmax[:], iota14f[:], OP.is_ge, OP.mult
            )
            idxi = workp.tile([1, 1], I32)
            with nc.allow_low_precision(reason="argmax index sum is exact in i32"):
                nc.vector.tensor_reduce(idxi[:], dotj[:], AX.X, OP.add)
            wimg = constp.tile([S, 3 * S], BF16)
            with nc.sync.register("ridx") as ridx:
                nc.sync.reg_load(ridx, idxi[0:1, 0:1])
                off = nc.sync.snap(ridx)
                nc.sync.dma_start(
                    wimg[:], imgs_d[bass.ds(off, 1), :, :].squeeze(0)
                )

            # ---- rounded box coords cs = rint(256 * X[0:4]) (vector)
            cs = constp.tile([128, 4], F32)
            nc.vector.tensor_scalar(cs[:], xb[:, 0:4], 256.0, MAGIC, OP.mult, OP.add)
            nc.vector.tensor_scalar(cs[:], cs[:], MAGIC, None, OP.subtract)

            # ---- per-partition scalars for the distance grids (vector)
            boxc64 = constp.tile([128, 1], F32)
            nc.vector.scalar_tensor_tensor(
                boxc64[:], cs[:, 3:4], cs[:, 2:3], c64[:], OP.subtract, OP.mult
            )
            boxr64 = constp.tile([128, 1], F32)
            nc.vector.scalar_tensor_tensor(
                boxr64[:], cs[:, 1:2], cs[:, 0:1], c64[:], OP.subtract, OP.mult
            )
            cs0r0 = constp.tile([128, 1], F32)
            nc.vector.tensor_tensor(cs0r0[:], cs[:, 0:1], xb[:, 19:20], OP.subtract)
            y1mp = workp.tile([128, 1], F32)
            nc.vector.tensor_tensor(y1mp[:], cs[:, 2:3], iota_pf[:], OP.subtract)
            y1mp128 = workp.tile([128, 1], F32)
            nc.vector.tensor_scalar(y1mp128[:], y1mp[:], 128.0, None, OP.subtract)

            # ---- column distance squares + one big exp (scalar)
            # dcsq_t[p,j] = (boxc64*j + y1 - p - 128 t)^2 = (src_c[j] - c)^2
            dcsq = workp.tile([128, 2, 64], F32)
            nc.scalar.activation(
                dcsq[:, 0, :], iota64f[:], AF.Square, bias=y1mp[:], scale=boxc64[:]
            )
            nc.scalar.activation(
                dcsq[:, 1, :], iota64f[:], AF.Square, bias=y1mp128[:], scale=boxc64[:]
            )
            ec = workp.tile([128, 2, 64], F32)
            nc.scalar.activation(ec[:], dcsq[:], AF.Exp, scale=-1.0)

            # ---- row exp table ErT[i, r] = exp(-(src_r[i] - (r0+r))^2)
            svecR = workp.tile([64, 1], F32)
            nc.scalar.activation(
                svecR[:], iota_pf[0:64, :], AF.Identity,
                bias=cs0r0[0:64, :], scale=boxr64[0:64, :],
            )
            drT2 = workp.tile([64, 64], F32)
            nc.scalar.activation(
                drT2[:], iota64f[0:64, :], AF.Square, bias=svecR[:], scale=-1.0
            )
            ErT = constp.tile([64, 64], BF16)
            nc.scalar.activation(ErT[:], drT2[:], AF.Exp, scale=-1.0)

            # ---- column-inside masks for both tiles (vector, before Ac)
            lt2 = workp.tile([128, 2], F32)
            nc.vector.tensor_scalar(lt2[:], iotap2[:], cs[:, 3:4], None, OP.is_lt)
            colp2 = workp.tile([128, 2], F32)
            nc.vector.scalar_tensor_tensor(
                colp2[:], iotap2[:], cs[:, 2:3], lt2[:], OP.is_ge, OP.mult
            )

            # ---- column softmax normalize + Ac tiles (vector)
            zc2 = workp.tile([128, 2], F32)
            nc.vector.tensor_reduce(zc2[:], ec[:], AX.X, OP.add)
            rzc2 = workp.tile([128, 2], F32)
            nc.vector.tensor_scalar(rzc2[:], zc2[:], 1e-30, None, OP.add)
            nc.vector.reciprocal(rzc2[:], rzc2[:])
            Ac_all = workp.tile([128, 2, 64], BF16)
            nc.vector.tensor_scalar(
                Ac_all[:, 0, :], ec[:, 0, :], rzc2[:, 0:1], colp2[:, 0:1],
                OP.mult, OP.mult,
            )
            nc.vector.tensor_scalar(
                Ac_all[:, 1, :], ec[:, 1, :], rzc2[:, 1:2], colp2[:, 1:2],
                OP.mult, OP.mult,
            )
            # ONE transpose for both tiles: [128,(t,j)] -> [(t,j),c]
            acTw_ps = ps_a.tile([128, 128], BF16, tag="acTw", name="acTwps")
            nc.tensor.transpose(
                acTw_ps[:], Ac_all[:].rearrange("p a b -> p (a b)"), idbf[:]
            )
            AcT = constp.tile([64, 256], BF16)
            nc.vector.tensor_copy(AcT[:, 0:128], acTw_ps[0:64, :])
            nc.scalar.copy(AcT[:, 128:256], acTw_ps[64:128, :])

            # ---- row normalizer: row AND column via two tiny PE matmuls
            zr_ps = ps_z.tile([1, 64], F32, tag="zrow")
            nc.tensor.matmul(zr_ps[:], ones64_bf[:], ErT[:])
            zrc_ps = ps_z.tile([64, 1], F32, tag="zcol", name="zrcps")
            nc.tensor.matmul(zrc_ps[:], ErT[:], ones64_bf[:])
            # -Zr row in bf16 (scalar: Identity scale=-1 + copy)
            rowzn = constp.tile([1, 128], BF16)
            nc.scalar.activation(rowzn[0:1, 0:64], zr_ps[:], AF.Identity, scale=-1.0)
            nc.scalar.copy(rowzn[0:1, 64:128], rowzn[0:1, 0:64])
            rzrc = workp.tile([128, 1], F32)
            nc.vector.tensor_scalar(rzrc[0:64, :], zrc_ps[:], 1e-30, None, OP.add)
            nc.vector.reciprocal(rzrc[0:64, :], rzrc[0:64, :])
            nc.vector.tensor_copy(rzrc[64:128, :], rzrc[0:64, :])

            # ---- validity / colin row / z' column (vector)
            bxbc = workp.tile([128, 1], F32)
            nc.vector.tensor_tensor(bxbc[:], boxr64[:], boxc64[:], OP.mult)
            valid = constp.tile([128, 1], F32)
            nc.vector.tensor_scalar(valid[:], bxbc[:], 0.0, None, OP.is_gt)
            cl = workp.tile([1, 256], F32)
            nc.vector.tensor_scalar(cl[:], iota256f[:], cs[0:1, 3:4], None, OP.is_lt)
            colin_row = constp.tile([1, 256], BF16)
            nc.vector.scalar_tensor_tensor(
                colin_row[:], iota256f[:], cs[0:1, 2:3], cl[:], OP.is_ge, OP.mult
            )
            cs1r0 = constp.tile([128, 1], F32)
            nc.vector.tensor_tensor(cs1r0[:], cs[:, 1:2], xb[:, 19:20], OP.subtract)
            ltr = workp.tile([128, 1], F32)
            nc.vector.tensor_scalar(ltr[:], pm64[:], cs1r0[:], None, OP.is_lt)
            rv = workp.tile([128, 1], F32)
            nc.vector.scalar_tensor_tensor(
                rv[:], pm64[:], cs0r0[:], ltr[:], OP.is_ge, OP.mult
            )
            rvv = workp.tile([128, 1], F32)
            nc.vector.tensor_tensor(rvv[:], rv[:], valid[:], OP.mult)
            z128 = constp.tile([128, 1], F32)
            nc.vector.tensor_tensor(z128[:], rvv[:], rzrc[:], OP.mult)

            # ---- correction outer products pre-accumulated into eR PSUM
            er_c_ps = ps_c.tile([64, 256], F32, tag="rc", name="erc")
            nc.tensor.matmul(
                er_c_ps[:], rowzn[0:1, 0:64], colin_row[:],
                start=True, stop=False, skip_group_check=True,
            )
            er_ab_ps = ps_r.tile([128, 256], F32, tag="rab", name="erab")
            nc.tensor.matmul(
                er_ab_ps[:], rowzn[:], colin_row[:],
                start=True, stop=False, skip_group_check=True,
            )

            # ---- t1[ch][j, r] = sum_i wimg[i, (ch,j)] * ErT[i, r]
            # (ch2 first so the res_c branch starts earliest)
            t1_ps = ps_t1.tile([64, 192], F32, tag="t1")
            for ch in (2, 0, 1):
                nc.tensor.matmul(
                    t1_ps[:, 64 * ch : 64 * (ch + 1)],
                    wimg[:, 64 * ch : 64 * (ch + 1)],
                    ErT[:],
                )
            t1all = constp.tile([64, 192], BF16)
            nc.scalar.copy(t1all[:, 128:192], t1_ps[:, 128:192])
            nc.vector.tensor_copy(t1all[:, 0:128], t1_ps[:, 0:128])

            # ---- eR matmuls accumulate on top of the corrections
            nc.tensor.matmul(
                er_c_ps[:], t1all[:, 128:192], AcT[:],
                start=False, stop=True, skip_group_check=True,
            )
            nc.tensor.matmul(
                er_ab_ps[:], t1all[:, 0:128], AcT[:],
                start=False, stop=True, skip_group_check=True,
            )

            # ---- single-op tails: res = z' * P + valid
            res_c = outp.tile([64, 256], F32)
            nc.scalar.activation(
                res_c[:], er_c_ps[:], AF.Identity,
                bias=valid[0:64, :], scale=z128[0:64, :],
            )
            res_ab = outp.tile([128, 256], F32)
            nc.vector.tensor_scalar(
                res_ab[:], er_ab_ps[:], z128[:], valid[:], OP.mult, OP.add
            )

            # ---- output DMAs on the two HWDGE rings in parallel
            nc.scalar.dma_start(out_d[2, :, :], res_c[:])
            nc.sync.dma_start(
                out_d[0:2, :, :].rearrange("a b c -> (a b) c"), res_ab[:]
            )

    nc.compile()
    return nc


_CACHE = {}


def get_nc():
    if "nc" not in _CACHE:
        _CACHE["nc"] = build_nc()
    return _CACHE["nc"]


def make_in_maps(X, images):
    X = np.ascontiguousarray(np.asarray(X, np.float32))
    images = np.ascontiguousarray(np.asarray(images, np.float32))
    # layout/dtype prep only: [14,4,64,64] f32 -> [14, 64(i), 3*64(ch,j)] bf16
    imgs_gt = np.ascontiguousarray(
        images[:, 0:3].transpose(0, 2, 1, 3).reshape(N_IMG, S, 3 * S)
    ).astype(ml_dtypes.bfloat16)
    in_maps = []
    for c in range(N_CORES):
        pic, rb = divmod(c, 4)
        xm = np.zeros((1, 24), np.float32)
        xm[0, :19] = X[pic, 0]
        xm[0, 19] = float(RB * rb)
        in_maps.append({"xmeta": np.tile(xm, (128, 1)), "imgs": imgs_gt})
    return in_maps


def assemble(results):
    out = np.empty((2, 3, H, H), np.float32)
    for c in range(N_CORES):
        pic, rb = divmod(c, 4)
        out[pic, :, RB * rb : RB * (rb + 1), :] = results[c]["out"]
    return out


def _axon_reset():
    try:
        import ctypes

        import jax

        jax.devices()
        ctypes.CDLL("/opt/axon/libaxon_pjrt.so").axon_reset()
    except Exception:
        pass


def kernel(X, images):
    nc = get_nc()
    in_maps = make_in_maps(X, images)
    try:
        res = run_bass_kernel_spmd(nc, in_maps, list(range(N_CORES)))
    except Exception:
        # the axon terminal can be left in a bad state by earlier failed
        # runs (LoadExecutable errors); reset and retry once
        _axon_reset()
        res = run_bass_kernel_spmd(nc, in_maps, list(range(N_CORES)))
    return assemble(res.results)


# revision 37
# speedup vs baseline: 1.0928x; 1.0928x over previous
"""Trainium2 Bass kernel for the emoji-box decoder problem (optimized v11).

Math: per picture, softmax(-d2) over emoji pixels is separable:
softmax_r (x) softmax_c.  With A = t1^T AcT the unnormalized row sum
(AcT column-normalized and column-masked), the output satisfies

    res = z' * (A - Zr (x) colin) + valid
    z'[p] = rowin[p] * (1/Zr[p])        (valid is implied by the row
                                         and column masks; the +valid
                                         bias term carries the rest)

because res = valid*(rowin*colin*(R-1) + 1) = where(valid,
where(inside, R, 1), 0).  The -Zr (x) colin term is ACCUMULATED INTO
THE SAME PSUM BANK as the eR matmul via a k=1 outer-product matmul
(lhsT = -Zr row in bf16 straight from a minus-ones matmul, rhs = colin
row), so the whole blend collapses to ONE fused multiply-add per
output group (tensor_scalar on vector for ch0/1, Identity-activation
with scale/bias pointers on scalar for ch2).  Zr is produced both as
a row (minus-ones^T ErT) and as a column (ErT^T ones) by two tiny PE
matmuls off the same ErT table.  Both Ac tiles are transposed in ONE
[128,128] PE pass.

Engine split respecting TRN2 ISA limits (Pool supports no float
elementwise at all; scalar does fused affine activations with
per-partition scale/bias pointers — Square(scale*x+bias) builds the
distance grids directly from an iota):
  vector: argmax, box coords, masks, softmax normalize, Ac tiles,
          AcT/-Zr/1/Zr psum copies, z' chain, res_ab tail
  scalar: distance-grid Squares, one big exp, svecR/ErT chain,
          t1 psum copies, res_c tail
  gpsimd: iotas / memsets / identity matrix only
  PE: one transpose, Zr row+col matmuls, correction outers, t1
      (ch2 in its own PSUM bank first), eR (c before ab)

Emoji selection: argmax of logits -> sync register -> dynamic-offset
DRAM->SBUF DMA gather of the selected emoji (bf16, i-major layout so
each partition reads one contiguous 384B chunk).  The two output DMAs
ride the scalar and sync HWDGE rings in parallel.

Sharding: 8 cores = 2 pictures x 4 row-blocks of 64 canvas rows.
"""

import sys

import numpy as np

if "/opt/trn_rl_repo" not in sys.path:
    sys.path.insert(0, "/opt/trn_rl_repo")

import ml_dtypes

import concourse.bacc as bacc
import concourse.bass as bass
import concourse.mybir as mybir
import concourse.tile as tile
from concourse.bass_utils import run_bass_kernel_spmd


def _ensure_ntff_hook():
    """The image's antenv package lacks axon_hooks, so trn_boot's NTFF
    profile hook install degrades silently and run_bass_kernel_spmd
    crashes on `from antenv.axon_hooks import ...` when trace=True.
    Provide the module and install the ctypes hook ourselves."""
    import types

    try:
        from antenv.axon_hooks import get_axon_ntff_profile_hook  # noqa: F401

        return
    except ImportError:
        pass
    mod = types.ModuleType("antenv.axon_hooks")
    _hook = [None]
    mod.set_axon_ntff_profile_hook = lambda h: _hook.__setitem__(0, h)
    mod.get_axon_ntff_profile_hook = lambda: _hook[0]
    try:
        import antenv

        sys.modules["antenv.axon_hooks"] = mod
        antenv.axon_hooks = mod
        from trn_agent_boot.trn_boot import _ntff_profile_via_ctypes

        hook = _ntff_profile_via_ctypes("/opt/axon/libaxon_pjrt.so")
        if hook is not None:
            mod.set_axon_ntff_profile_hook(hook)
    except Exception:
        pass


_ensure_ntff_hook()

F32 = mybir.dt.float32
BF16 = mybir.dt.bfloat16
I32 = mybir.dt.int32
AF = mybir.ActivationFunctionType
OP = mybir.AluOpType
AX = mybir.AxisListType

MAGIC = 8388608.0  # 2**23; x + MAGIC - MAGIC == rint(x) for 0 <= x < 2**22

N_CORES = 8
H = 256
S = 64
N_IMG = 14
RB = 64  # canvas rows per core


def build_nc():
    nc = bacc.Bacc("TRN2", target_bir_lowering=False, debug=False)

    xmeta_d = nc.dram_tensor("xmeta", [128, 24], F32, kind="ExternalInput")
    imgs_d = nc.dram_tensor("imgs", [N_IMG, S, 3 * S], BF16, kind="ExternalInput")
    out_d = nc.dram_tensor("out", [3, RB, H], F32, kind="ExternalOutput")

    with tile.TileContext(nc) as tc:
        with (
            tc.tile_pool(name="constp", bufs=1) as constp,
            tc.tile_pool(name="workp", bufs=2) as workp,
            tc.tile_pool(name="outp", bufs=1) as outp,
            tc.tile_pool(name="ps_a", bufs=1, space="PSUM") as ps_a,
            tc.tile_pool(name="ps_z", bufs=1, space="PSUM") as ps_z,
            tc.tile_pool(name="ps_t1", bufs=1, space="PSUM") as ps_t1,
            tc.tile_pool(name="ps_t2", bufs=1, space="PSUM") as ps_t2,
            tc.tile_pool(name="ps_r", bufs=1, space="PSUM") as ps_r,
            tc.tile_pool(name="ps_c", bufs=1, space="PSUM") as ps_c,
        ):
            # ---- single input DMA on the sync HWDGE ring
            xb = constp.tile([128, 24], F32)
            nc.sync.dma_start(xb[:], xmeta_d[:])

            # ---- warm the scalar-engine activation table early so the
            # ~1.3us ACT_TABLE_LOAD overlaps the input DMAs
            warm = workp.tile([1, 1], F32)
            nc.gpsimd.memset(warm[:], 0.0)
            warm2 = workp.tile([1, 1], F32)
            nc.scalar.activation(warm2[:], warm[:], AF.Exp)

            # ---- constants / iotas (gpsimd+vector, overlap the DMAs)
            iota_pi = constp.tile([128, 1], I32)
            nc.gpsimd.iota(iota_pi[:], pattern=[[1, 1]], base=0, channel_multiplier=1)
            iota_pf = constp.tile([128, 1], F32)
            nc.vector.tensor_copy(iota_pf[:], iota_pi[:])
            iota14i = constp.tile([1, N_IMG], I32)
            nc.gpsimd.iota(iota14i[:], pattern=[[1, N_IMG]], base=0, channel_multiplier=0)
            iota14f = constp.tile([1, N_IMG], F32)
            nc.vector.tensor_copy(iota14f[:], iota14i[:])
            iota64i = constp.tile([128, 64], I32)
            nc.gpsimd.iota(iota64i[:], pattern=[[1, 64]], base=0, channel_multiplier=0)
            iota64f = constp.tile([128, 64], F32)
            nc.vector.tensor_copy(iota64f[:], iota64i[:])
            iota256i = constp.tile([1, 256], I32)
            nc.gpsimd.iota(iota256i[:], pattern=[[1, 256]], base=0, channel_multiplier=0)
            iota256f = constp.tile([1, 256], F32)
            nc.vector.tensor_copy(iota256f[:], iota256i[:])
            # iotap2[p, t] = p + 128*t  (both canvas-column tiles at once)
            iotap2i = constp.tile([128, 2], I32)
            nc.gpsimd.iota(iotap2i[:], pattern=[[128, 2]], base=0, channel_multiplier=1)
            iotap2 = constp.tile([128, 2], F32)
            nc.vector.tensor_copy(iotap2[:], iotap2i[:])
            # pm64[p] = p % 64 via two sliced iotas (channel index is
            # relative to the AP start)
            pm64i = constp.tile([128, 1], I32)
            nc.gpsimd.iota(pm64i[0:64, :], pattern=[[1, 1]], base=0, channel_multiplier=1)
            nc.gpsimd.iota(pm64i[64:128, :], pattern=[[1, 1]], base=0, channel_multiplier=1)
            pm64 = constp.tile([128, 1], F32)
            nc.vector.tensor_copy(pm64[:], pm64i[:])
            c64 = constp.tile([128, 1], F32)
            nc.vector.memset(c64[:], 1.0 / 64.0)
            ones64_bf = constp.tile([64, 1], BF16)
            nc.gpsimd.memset(ones64_bf[:], 1.0)
            mones64_bf = constp.tile([64, 1], BF16)
            nc.gpsimd.memset(mones64_bf[:], -1.0)
            onebf = workp.tile([128, 128], BF16)
            nc.gpsimd.memset(onebf[:], 1.0)
            idbf = constp.tile([128, 128], BF16)
            nc.gpsimd.affine_select(
                idbf[:],
                onebf[:],
                pattern=[[1, 128]],
                compare_op=OP.is_equal,
                fill=0.0,
                base=0,
                channel_multiplier=-1,
            )

            # ================= after xmeta arrives =================
            # ---- emoji index chain (vector) -> sync register ->
            # SBUF->SBUF gather from the prefetched table
            rmax = workp.tile([1, 1], F32)
            nc.vector.tensor_reduce(rmax[:], xb[0:1, 5:19], AX.X, OP.max)
            cs = constp.tile([128, 4], F32)
            nc.vector.tensor_scalar(cs[:], xb[:, 0:4], 256.0, MAGIC, OP.mult, OP.add)
            nc.vector.tensor_scalar(cs[:], cs[:], MAGIC, None, OP.subtract)
            dotj = workp.tile([1, N_IMG], I32)
            nc.vector.scalar_tensor_tensor(
                dotj[:], xb[0:1, 5:19], rmax[:], iota14f[:], OP.is_ge, OP.mult
            )
            idxi = workp.tile([1, 1], I32)
            with nc.allow_low_precision(reason="argmax index sum is exact in i32"):
                nc.vector.tensor_reduce(idxi[:], dotj[:], AX.X, OP.add)
            wimg = constp.tile([S, 3 * S], BF16)
            with nc.sync.register("ridx") as ridx:
                nc.sync.reg_load(ridx, idxi[0:1, 0:1])
                off = nc.sync.snap(ridx)
                nc.sync.dma_start(
                    wimg[:], imgs_d[bass.ds(off, 1), :, :].squeeze(0)
                )

            # ---- per-partition scalars for the distance grids (vector)
            boxc64 = constp.tile([128, 1], F32)
            nc.vector.scalar_tensor_tensor(
                boxc64[:], cs[:, 3:4], cs[:, 2:3], c64[:], OP.subtract, OP.mult
            )
            boxr64 = constp.tile([128, 1], F32)
            nc.vector.scalar_tensor_tensor(
                boxr64[:], cs[:, 1:2], cs[:, 0:1], c64[:], OP.subtract, OP.mult
            )
            cs0r0 = constp.tile([128, 1], F32)
            nc.vector.tensor_tensor(cs0r0[:], cs[:, 0:1], xb[:, 19:20], OP.subtract)
            y1mp = workp.tile([128, 1], F32)
            nc.vector.tensor_tensor(y1mp[:], cs[:, 2:3], iota_pf[:], OP.subtract)
            y1mp128 = workp.tile([128, 1], F32)
            nc.vector.tensor_scalar(y1mp128[:], y1mp[:], 128.0, None, OP.subtract)

            # ---- column distance squares (scalar fused Square) + one
            # big exp (scalar)
            dcsq = workp.tile([128, 2, 64], F32)
            nc.scalar.activation(
                dcsq[:, 0, :], iota64f[:], AF.Square, bias=y1mp[:], scale=boxc64[:]
            )
            nc.scalar.activation(
                dcsq[:, 1, :], iota64f[:], AF.Square, bias=y1mp128[:], scale=boxc64[:]
            )
            ec = workp.tile([128, 2, 64], F32)
            nc.scalar.activation(ec[:], dcsq[:], AF.Exp, scale=-1.0)
            # colin row early (vector): feeds the correction outers
            cl = workp.tile([1, 256], F32)
            nc.vector.tensor_scalar(cl[:], iota256f[:], cs[0:1, 3:4], None, OP.is_lt)
            colin_row = constp.tile([1, 256], BF16)
            nc.vector.scalar_tensor_tensor(
                colin_row[:], iota256f[:], cs[0:1, 2:3], cl[:], OP.is_ge, OP.mult
            )

            # ---- row exp table ErT[i, r] = exp(-(src_r[i] - (r0+r))^2)
            svecR = workp.tile([64, 1], F32)
            nc.scalar.activation(
                svecR[:], iota_pf[0:64, :], AF.Identity,
                bias=cs0r0[0:64, :], scale=boxr64[0:64, :],
            )
            drT2 = workp.tile([64, 64], F32)
            nc.scalar.activation(
                drT2[:], iota64f[0:64, :], AF.Square, bias=svecR[:], scale=-1.0
            )
            ErT = constp.tile([64, 64], BF16)
            nc.scalar.activation(ErT[:], drT2[:], AF.Exp, scale=-1.0)

            # ---- column-inside masks for both tiles (vector, before Ac)
            lt2 = workp.tile([128, 2], F32)
            nc.vector.tensor_scalar(lt2[:], iotap2[:], cs[:, 3:4], None, OP.is_lt)
            colp2 = workp.tile([128, 2], F32)
            nc.vector.scalar_tensor_tensor(
                colp2[:], iotap2[:], cs[:, 2:3], lt2[:], OP.is_ge, OP.mult
            )

            # ---- column softmax normalize + Ac tiles (vector)
            zc2 = workp.tile([128, 2], F32)
            nc.vector.tensor_reduce(zc2[:], ec[:], AX.X, OP.add)
            rzc2 = workp.tile([128, 2], F32)
            nc.vector.tensor_scalar(rzc2[:], zc2[:], 1e-30, None, OP.add)
            nc.vector.reciprocal(rzc2[:], rzc2[:])
            Ac_all = workp.tile([128, 2, 64], BF16)
            nc.vector.tensor_scalar(
                Ac_all[:, 0, :], ec[:, 0, :], rzc2[:, 0:1], colp2[:, 0:1],
                OP.mult, OP.mult,
            )
            nc.vector.tensor_scalar(
                Ac_all[:, 1, :], ec[:, 1, :], rzc2[:, 1:2], colp2[:, 1:2],
                OP.mult, OP.mult,
            )
            # ONE transpose for both tiles: [128,(t,j)] -> [(t,j),c]
            acTw_ps = ps_a.tile([128, 128], BF16, tag="acTw", name="acTwps")
            nc.tensor.transpose(
                acTw_ps[:], Ac_all[:].rearrange("p a b -> p (a b)"), idbf[:]
            )
            AcT = constp.tile([64, 256], BF16)
            nc.vector.tensor_copy(AcT[:, 0:128], acTw_ps[0:64, :])
            nc.vector.tensor_copy(AcT[:, 128:256], acTw_ps[64:128, :])

            # ---- row normalizer: -row and +column via two PE matmuls
            zr_ps = ps_z.tile([1, 64], F32, tag="zrow")
            nc.tensor.matmul(zr_ps[:], mones64_bf[:], ErT[:])
            zrc_ps = ps_z.tile([64, 1], F32, tag="zcol", name="zrcps")
            nc.tensor.matmul(zrc_ps[:], ErT[:], ones64_bf[:])
            # -Zr row in bf16 (vector copy + duplicate)
            rowzn = constp.tile([1, 128], BF16)
            nc.vector.tensor_copy(rowzn[0:1, 0:64], zr_ps[:])
            nc.vector.tensor_copy(rowzn[0:1, 64:128], rowzn[0:1, 0:64])
            rzrc = workp.tile([128, 1], F32)
            nc.vector.tensor_scalar(rzrc[0:64, :], zrc_ps[:], 1e-30, None, OP.add)
            nc.vector.reciprocal(rzrc[0:64, :], rzrc[0:64, :])
            nc.vector.tensor_copy(rzrc[64:128, :], rzrc[0:64, :])

            # ---- z' column (vector; valid folded out)
            cs1r0 = constp.tile([128, 1], F32)
            nc.vector.tensor_tensor(cs1r0[:], cs[:, 1:2], xb[:, 19:20], OP.subtract)
            ltr = workp.tile([128, 1], F32)
            nc.vector.tensor_scalar(ltr[:], pm64[:], cs1r0[:], None, OP.is_lt)
            rv = workp.tile([128, 1], F32)
            nc.vector.scalar_tensor_tensor(
                rv[:], pm64[:], cs0r0[:], ltr[:], OP.is_ge, OP.mult
            )
            z128 = constp.tile([128, 1], F32)
            nc.vector.tensor_tensor(z128[:], rv[:], rzrc[:], OP.mult)
            # validity bias (only the non-tautological conditions)
            bxbc = workp.tile([128, 1], F32)
            nc.vector.tensor_tensor(bxbc[:], boxr64[:], boxc64[:], OP.mult)
            valid = constp.tile([128, 1], F32)
            nc.vector.tensor_scalar(valid[:], bxbc[:], 0.0, None, OP.is_gt)

            # ---- correction outer products pre-accumulated into eR PSUM
            er_c_ps = ps_c.tile([64, 256], F32, tag="rc", name="erc")
            nc.tensor.matmul(
                er_c_ps[:], rowzn[0:1, 0:64], colin_row[:],
                start=True, stop=False, skip_group_check=True,
            )
            er_ab_ps = ps_r.tile([128, 256], F32, tag="rab", name="erab")
            nc.tensor.matmul(
                er_ab_ps[:], rowzn[:], colin_row[:],
                start=True, stop=False, skip_group_check=True,
            )

            # ---- t1[ch][j, r] = sum_i wimg[i, (ch,j)] * ErT[i, r]
            # (ch2 first, in its own PSUM bank, so its copy never waits
            # on the ch0/ch1 matmuls)
            t1c_ps = ps_t1.tile([64, 64], F32, tag="t1c")
            nc.tensor.matmul(t1c_ps[:], wimg[:, 128:192], ErT[:])
            t1ab_ps = ps_t2.tile([64, 128], F32, tag="t1ab")
            for ch in (0, 1):
                nc.tensor.matmul(
                    t1ab_ps[:, 64 * ch : 64 * (ch + 1)],
                    wimg[:, 64 * ch : 64 * (ch + 1)],
                    ErT[:],
                )
            t1all = constp.tile([64, 192], BF16)
            nc.scalar.copy(t1all[:, 128:192], t1c_ps[:])
            nc.scalar.copy(t1all[:, 0:128], t1ab_ps[:])

            # ---- eR matmuls accumulate on top of the corrections
            nc.tensor.matmul(
                er_c_ps[:], t1all[:, 128:192], AcT[:],
                start=False, stop=True, skip_group_check=True,
            )
            nc.tensor.matmul(
                er_ab_ps[:], t1all[:, 0:128], AcT[:],
                start=False, stop=True, skip_group_check=True,
            )

            # ---- tails res = z'*P + valid; res_c on scalar feeding the
            # scalar ring, res_ab on vector feeding the sync ring
            res_c = outp.tile([64, 256], F32)
            nc.scalar.activation(
                res_c[:], er_c_ps[:], AF.Identity,
                bias=valid[0:64, :], scale=z128[0:64, :],
            )
            nc.scalar.dma_start(out_d[2, :, :], res_c[:])
            res_ab = outp.tile([128, 256], F32)
            nc.vector.tensor_scalar(
                res_ab[:], er_ab_ps[:], z128[:], valid[:], OP.mult, OP.add
            )
            nc.sync.dma_start(
                out_d[0:2, :, :].rearrange("a b c -> (a b) c"), res_ab[:]
            )

    nc.compile()
    return nc


_CACHE = {}


def get_nc():
    if "nc" not in _CACHE:
        _CACHE["nc"] = build_nc()
    return _CACHE["nc"]


def make_in_maps(X, images):
    X = np.ascontiguousarray(np.asarray(X, np.float32))
    images = np.ascontiguousarray(np.asarray(images, np.float32))
    # layout/dtype prep only: [14,4,64,64] f32 -> [14, 64(i), 3*64(ch,j)] bf16
    imgs_it = np.ascontiguousarray(
        images[:, 0:3].transpose(0, 2, 1, 3).reshape(N_IMG, S, 3 * S)
    ).astype(ml_dtypes.bfloat16)
    in_maps = []
    for c in range(N_CORES):
        pic, rb = divmod(c, 4)
        xm = np.zeros((1, 24), np.float32)
        xm[0, :19] = X[pic, 0]
        xm[0, 19] = float(RB * rb)
        in_maps.append({"xmeta": np.tile(xm, (128, 1)), "imgs": imgs_it})
    return in_maps


def assemble(results):
    out = np.empty((2, 3, H, H), np.float32)
    for c in range(N_CORES):
        pic, rb = divmod(c, 4)
        out[pic, :, RB * rb : RB * (rb + 1), :] = results[c]["out"]
    return out


def _axon_reset():
    try:
        import ctypes

        import jax

        jax.devices()
        ctypes.CDLL("/opt/axon/libaxon_pjrt.so").axon_reset()
    except Exception:
        pass


def kernel(X, images):
    nc = get_nc()
    in_maps = make_in_maps(X, images)
    try:
        res = run_bass_kernel_spmd(nc, in_maps, list(range(N_CORES)))
    except Exception:
        # the axon terminal can be left in a bad state by earlier failed
        # runs (LoadExecutable errors); reset and retry once
        _axon_reset()
        res = run_bass_kernel_spmd(nc, in_maps, list(range(N_CORES)))
    return assemble(res.results)
